# revision 94
# baseline (speedup 1.0000x reference)
"""BandSplit (per-band LayerNorm + Linear) on 8 Trainium2 NeuronCores.

Strategy: data-parallel over batch (B=8 -> one batch element per core).
Per core, all 45 bands are computed with the feature (contraction) axis on
SBUF partitions:

  z_i = W_i'^T (rstd_i .* X_i)  -  s_i (x) (m_i .* rstd_i)   (+ b_i'' on host)

where per band i:
  X_i   : (d_i, T) slice of the host-prepermuted feature-major fp16 input,
  W_i'  : gamma-folded fp16 weights (d_i, E),
  s_i   : column sums of W_i' (length E),
  m_i, rstd_i : per-timestep LayerNorm stats (length T).

Dataflow, pipelined over 6 feature chunks (no band straddles a chunk, so
each chunk's bands finish while later chunks still stream from HBM):
  1. stats: per 128-row tile, squares (GPSIMD/DVE/ACT) and two fp8-indicator
     matmuls accumulate per-band sum / sum-of-squares into PSUM,
  2. LN params: m = s1/d, var = s2/d - m^2, rstd = 1/sqrt(var+eps) (DVE/ACT),
  3. rstd pre-scale: RST_g = INDT_g @ r16 broadcasts each band's rstd row
     over its feature rows (PE), xs_g = X_g * RST_g (one DVE multiply),
  4. per band: K-tiled matmul W'^T xs + one K=1 accumulating matmul for the
     rank-1 mean correction (-s_i x (m_i*rstd_i)), ACT-copy eviction to fp16,
     paired-band output DMA.
The emission order software-pipelines chunks so the in-order engine queues
never stall on another chunk's serial LN-params chain. Inputs ride the ACT
HWDGE ring, outputs the SP ring. fp16 matmuls accumulate in fp32 PSUM; all
stats math is fp32.

Output leaves the device as (band, E, T) fp16 per batch element (contiguous
DMA) and is transposed/cast to (B, band, T, E) fp32 on the host.
"""

import os

import numpy as np

# ---- problem constants (hardcoded; kernel.py must be self-contained) ----
WIDTHS = [16] * 32 + [32] * 8 + [64] * 4 + [1]
_EF = np.concatenate([[0], np.cumsum(WIDTHS)]).astype(int)  # freq edges
DS = [4 * w for w in WIDTHS]  # per-band feature dims
_ED = np.concatenate([[0], np.cumsum(DS)]).astype(int)  # feature edges
NB = len(WIDTHS)  # 45 bands
B, C, F, T, E = 8, 2, 1025, 512, 128
DTOT = int(_ED[-1])  # 4100
NG = 33  # 128-row feature tiles
DPAD = NG * 128  # 4224
NBPAD = 64  # padded band count (stats partitions)
EPS = 1e-5
N_CORES = 8

_prog_cache = {}
LAST_RESULTS = None  # BassKernelResults of the last run (for test harness)


def _band_ktiles(i):
    """[(g, r0, r1)] tile/row spans covering features [_ED[i], _ED[i+1])."""
    lo, hi = int(_ED[i]), int(_ED[i + 1])
    out = []
    while lo < hi:
        g = lo // 128
        step = min(hi, (g + 1) * 128) - lo
        out.append((g, lo - g * 128, lo - g * 128 + step))
        lo += step
    return out


def _build_program():
    import concourse.mybir as mybir
    import concourse.tile as tile
    from concourse import bacc

    f16, f32 = mybir.dt.float16, mybir.dt.float32
    f8 = mybir.dt.float8e4
    nc = bacc.Bacc(
        "TRN2", target_bir_lowering=False, debug=False, num_devices=N_CORES
    )

    xh = nc.dram_tensor("xh", [NG, 128, T], f16, kind="ExternalInput").ap()
    wg = nc.dram_tensor("wg", [NG, 128, E], f16, kind="ExternalInput").ap()
    ind = nc.dram_tensor("ind", [NG, 128, NBPAD], f8, kind="ExternalInput").ap()
    indt = nc.dram_tensor("indt", [NBPAD, NG, 128], f8, kind="ExternalInput").ap()
    sneg = nc.dram_tensor("sneg", [1, NBPAD * E], f16, kind="ExternalInput").ap()
    invd = nc.dram_tensor("invd", [NBPAD, 1], f32, kind="ExternalInput").ap()
    y = nc.dram_tensor("y", [NB, E, T], f16, kind="ExternalOutput").ap()

    with tile.TileContext(nc) as tc:
        with (
            tc.tile_pool(name="big", bufs=1) as big,
            tc.tile_pool(name="sq", bufs=6) as sqp,
            tc.tile_pool(name="qs_ps", bufs=6, space="PSUM") as qp,
            tc.tile_pool(name="r_ps", bufs=2, space="PSUM") as rpp,
            tc.tile_pool(name="rb", bufs=10) as rp,
            tc.tile_pool(name="ob", bufs=8) as outp,
            tc.tile_pool(name="bstat", bufs=3) as bstat,
            tc.tile_pool(name="small", bufs=1) as small,
        ):
            # X split into chunk tiles so each is filled by ONE DMA (keeps
            # per-instruction sync-wait counts low and lets compute start
            # before the whole input has landed). Chunk boundaries are chosen
            # so no band's feature rows straddle a chunk: stats, LN params,
            # and outputs for each chunk's bands complete while later chunks
            # are still streaming in.
            XCHUNKS = ((0, 4), (4, 10), (10, 16), (16, 24), (24, 30), (30, NG))
            xtiles = [
                big.tile([128, g1 - g0, T], f16, tag=f"X{ci}", name=f"X{ci}")
                for ci, (g0, g1) in enumerate(XCHUNKS)
            ]

            def xt(g):
                for ci, (g0, g1) in enumerate(XCHUNKS):
                    if g0 <= g < g1:
                        return xtiles[ci], g - g0
                raise ValueError(g)

            W = big.tile([128, NG, E], f16, tag="W")
            IND = big.tile([128, NG, NBPAD], f8, tag="IND")
            INDT = big.tile([NBPAD, NG, 128], f8, tag="INDT")
            SNEG = small.tile([1, NBPAD * E], f16, tag="SNEG")
            INVD = small.tile([NBPAD, 1], f32, tag="INVD")
            # fp16 rstd rows for all bands; rhs of the per-g RST broadcast
            # matmuls (K=64: other chunks' rows hit zero indicator columns,
            # so memset once to keep them finite)
            R16 = small.tile([NBPAD, T], f16, tag="R16")
            nc.vector.memset(R16[:, :], 0.0)

            # ---- input DMAs; HWDGE drains mostly in issue order, so land
            # the small tensors phase A needs first, then stream X chunks
            nc.scalar.dma_start(out=IND[:, :, :], in_=ind.rearrange("g p n -> p g n"))
            nc.scalar.dma_start(out=INVD[:, :], in_=invd[:, :])
            for ci, (g0, g1) in enumerate(XCHUNKS):
                nc.scalar.dma_start(
                    out=xtiles[ci][:, :, :],
                    in_=xh[g0:g1].rearrange("g p t -> p g t"),
                )
                if ci == 0:
                    nc.scalar.dma_start(out=INDT[:, :, :], in_=indt[:, :, :])
                    nc.scalar.dma_start(out=SNEG[:, :], in_=sneg[:, :])
                elif ci == 1:
                    nc.scalar.dma_start(
                        out=W[:, :, :], in_=wg.rearrange("g p e -> p g e")
                    )

            epst = small.tile([NBPAD, 1], f32, tag="epst")
            nc.vector.memset(epst[:, :], EPS)

            # bands wholly contained in each chunk's g-range
            chunk_bands = [
                [
                    i for i in range(NB)
                    if all(g0 <= g < g1 for g, _, _ in _band_ktiles(i))
                ]
                for (g0, g1) in XCHUNKS
            ]
            assert sum(len(b) for b in chunk_bands) == NB

            # Per chunk: stats matmuls -> LN params -> rstd pre-scale of X ->
            # per-band matmul + mean correction -> eviction + output DMA.
            # Stats PSUM tiles share one rotation ("qs" tag) with the band
            # accumulators so all 6 PSUM banks of the pool stay busy. The
            # emission order below software-pipelines chunks: the in-order
            # engine queues see chunk c+1's stats ahead of chunk c's band
            # matmuls, so no engine stalls on another chunk's serial
            # LN-params chain.
            NCH = len(XCHUNKS)
            state = {}

            def emit_stats(c):
                g0, g1 = XCHUNKS[c]
                s1 = qp.tile([NBPAD, T], f32, tag="qs", name=f"s1_{c}")
                s2 = qp.tile([NBPAD, T], f32, tag="qs", name=f"s2_{c}")
                for g in range(g0, g1):
                    Xc, gl = xt(g)
                    sq = sqp.tile([128, T], f16, tag="sq", name="sq")
                    if g % 3 == 0:
                        nc.gpsimd.tensor_mul(sq[:, :], Xc[:, gl, :], Xc[:, gl, :])
                    elif g % 3 == 1 and g >= 12:
                        nc.vector.tensor_mul(sq[:, :], Xc[:, gl, :], Xc[:, gl, :])
                    else:
                        nc.scalar.square(out=sq[:, :], in_=Xc[:, gl, :])
                    nc.tensor.matmul(
                        s1[:, :], IND[:, g, :], Xc[:, gl, :],
                        start=(g == g0), stop=(g == g1 - 1),
                    )
                    nc.tensor.matmul(
                        s2[:, :], IND[:, g, :], sq[:, :],
                        start=(g == g0), stop=(g == g1 - 1),
                    )
                state[c] = {"s1": s1, "s2": s2}

            def emit_lnparams(c):
                # m = s1/d; var = s2/d - m^2; rstd = 1/sqrt(var+eps)
                # (full 64-row ops: engines need 32-aligned partition bases;
                # other chunks' rows are zeros here and only ever multiplied
                # by zero indicator columns later)
                st = state[c]
                bands = chunk_bands[c]
                lo, hi = bands[0], bands[-1] + 1
                s1, s2 = st["s1"], st["s2"]
                m = bstat.tile([NBPAD, T], f32, tag="m", name="m")
                msq = bstat.tile([NBPAD, T], f32, tag="msq", name="msq")
                var = bstat.tile([NBPAD, T], f32, tag="var", name="var")
                sd = bstat.tile([NBPAD, T], f32, tag="sd", name="sd")
                mr16 = bstat.tile([NBPAD, T], f16, tag="mr16", name="mr16")
                mr16f = bstat.tile(
                    [1, (hi - lo) * T], f16, tag="mr16f", name="mr16f"
                )
                r16 = bstat.tile([NBPAD, T], f16, tag="r16", name="r16")
                nc.vector.tensor_scalar_mul(m[:, :], s1[:, :], INVD[:, :])
                nc.vector.tensor_mul(msq[:, :], m[:, :], m[:, :])
                nc.vector.scalar_tensor_tensor(
                    out=var[:, :], in0=s2[:, :], scalar=INVD[:, :],
                    in1=msq[:, :],
                    op0=mybir.AluOpType.mult, op1=mybir.AluOpType.subtract,
                )
                nc.scalar.activation(
                    sd[:, :], var[:, :],
                    mybir.ActivationFunctionType.Sqrt, bias=epst[:, :],
                )
                with nc.allow_low_precision("rstd is consumed as fp16 anyway"):
                    nc.vector.reciprocal(r16[:, :], sd[:, :])
                # correction rows: (m * rstd) flattened onto partition 0
                nc.gpsimd.tensor_mul(mr16[:, :], m[:, :], r16[:, :])
                nc.sync.dma_start(out=mr16f[0:1, :], in_=mr16[lo:hi, :])
                st.update(r16=r16, mr16f=mr16f, lo=lo)

            def emit_rst(c):
                # pre-scale X by rstd: RST_g[f,t] = rstd_{band(f),t} via the
                # transposed indicator matmul, then xs_g = X_g * RST_g (DVE)
                g0, g1 = XCHUNKS[c]
                st = state[c]
                xs = {}
                for g in range(g0, g1):
                    Xc, gl = xt(g)
                    RST = rpp.tile([128, T], f32, tag="RST", name="RST")
                    nc.tensor.matmul(
                        RST[:, :], INDT[:, g, :], st["r16"][:, :],
                        start=True, stop=True,
                    )
                    xg16 = rp.tile([128, T], f16, tag="xs", name=f"xs_{g}")
                    nc.vector.scalar_tensor_tensor(
                        out=xg16[:, :], in0=RST[:, :], scalar=1.0,
                        in1=Xc[:, gl, :],
                        op0=mybir.AluOpType.mult, op1=mybir.AluOpType.mult,
                    )
                    xs[g] = xg16
                st["xs"] = xs

            def emit_bands(c):
                st = state[c]
                lo = st["lo"]
                bands = chunk_bands[c]
                pend = None  # (first_band, o2 tile) awaiting its pair
                for i in bands:
                    il = i - lo
                    q = qp.tile([128, T], f32, tag="qs", name=f"q_{i}")
                    for j, (g, r0, r1) in enumerate(_band_ktiles(i)):
                        nc.tensor.matmul(
                            q[:, :], W[r0:r1, g, :], st["xs"][g][r0:r1, :],
                            start=(j == 0), stop=False,
                        )
                    nc.tensor.matmul(
                        q[:, :],
                        SNEG[0:1, i * E:(i + 1) * E],
                        st["mr16f"][0:1, il * T:(il + 1) * T],
                        start=False, stop=True,
                    )
                    # adjacent bands share a double-wide staging tile and one
                    # output DMA (y rows are contiguous per band)
                    if pend is None:
                        o2 = outp.tile([128, 2, T], f16, tag="o", name="o")
                        nc.scalar.copy(out=o2[:, 0, :], in_=q[:, :])
                        if i == bands[-1]:
                            nc.sync.dma_start(out=y[i], in_=o2[:, 0, :])
                        else:
                            pend = (i, o2)
                    else:
                        i0, o2 = pend
                        nc.scalar.copy(out=o2[:, 1, :], in_=q[:, :])
                        nc.sync.dma_start(
                            out=y[i0:i0 + 2].rearrange("b p t -> p b t"),
                            in_=o2[:, :, :],
                        )
                        pend = None
                del state[c]

            emit_stats(0)
            emit_lnparams(0)
            emit_rst(0)
            for c in range(NCH):
                if c + 1 < NCH:
                    emit_stats(c + 1)
                    emit_lnparams(c + 1)
                emit_bands(c)
                if c + 1 < NCH:
                    emit_rst(c + 1)
    nc.compile()
    return nc


def _prep(x, gammas, betas, Ws, bs):
    x = np.asarray(x, dtype=np.float32)
    xg = np.zeros((B, DPAD, T), np.float16)
    for i in range(NB):
        fs, fe = int(_EF[i]), int(_EF[i + 1])
        xb = np.transpose(x[:, :, fs:fe, :, :], (0, 1, 4, 2, 3))  # b c r f t
        xg[:, _ED[i]:_ED[i + 1], :] = xb.reshape(B, DS[i], T).astype(np.float16)

    wgf = np.zeros((DPAD, E), np.float32)
    snegf = np.zeros((1, NBPAD * E), np.float32)
    b2 = np.zeros((NB, E), np.float32)
    for i in range(NB):
        Wp = np.asarray(Ws[i], np.float32) * np.asarray(gammas[i], np.float32)[:, None]
        wgf[_ED[i]:_ED[i + 1]] = Wp
        snegf[0, i * E:(i + 1) * E] = -Wp.sum(axis=0)
        b2[i] = np.asarray(betas[i], np.float32) @ Wp + np.asarray(bs[i], np.float32)

    import ml_dtypes
    indf = np.zeros((DPAD, NBPAD), ml_dtypes.float8_e4m3)
    invdf = np.ones((NBPAD, 1), np.float32)
    for i in range(NB):
        indf[_ED[i]:_ED[i + 1], i] = 1.0
        invdf[i, 0] = 1.0 / DS[i]
    indtf = np.ascontiguousarray(indf.T.reshape(NBPAD, NG, 128))

    return (
        np.ascontiguousarray(xg.reshape(B, NG, 128, T)),
        np.ascontiguousarray(wgf.astype(np.float16).reshape(NG, 128, E)),
        np.ascontiguousarray(indf.reshape(NG, 128, NBPAD)),
        indtf,
        snegf.astype(np.float16),
        invdf,
        b2,
    )


def kernel(x, gammas, betas, Ws, bs):
    global LAST_RESULTS
    from concourse.bass_utils import run_bass_kernel_spmd

    xg, wgt, indtile, indttile, snegt, invdt, b2 = _prep(x, gammas, betas, Ws, bs)

    if "nc" not in _prog_cache:
        _prog_cache["nc"] = _build_program()
    nc = _prog_cache["nc"]

    in_maps = [
        {
            "xh": xg[b], "wg": wgt, "ind": indtile, "indt": indttile,
            "sneg": snegt, "invd": invdt,
        }
        for b in range(B)
    ]
    trace = os.environ.get("BS_TRACE") == "1"
    res = run_bass_kernel_spmd(
        nc, in_maps, core_ids=list(range(N_CORES)), trace=trace
    )
    LAST_RESULTS = res

    yy = np.stack([res.results[b]["y"] for b in range(B)])  # (B, NB, E, T)
    out = np.ascontiguousarray(np.swapaxes(yy, 2, 3)).astype(np.float32)
    if np.any(b2 != 0.0):
        out = out + b2[None, :, None, :]
    return out
